# revision 50
# baseline (speedup 1.0000x reference)
"""Trainium2 Bass kernel for nn_MoEConnectionProcessor (v3).

Math (per row b, D=64, K=26):
  masks from tier (0=local,1=func,2=dist)
  agg_l = masked_mean(ns, tier==0); h_local = tanh([cs,agg_l]@W_local)
  msg = relu(ns@W2 + cs@W1 + b_msg) per (b,k); agg_f = masked_mean(msg, tier==1)
  h = tanh([cs,agg_f]@W_upd); 3x Euler: h += .1*tanh(h@W_fcnf)
  agg_d = masked_mean(ns, tier==2); h_dist=cs; 3x: h += .1*tanh([h,agg_d]@W_dcnf)
  gates = softmax(relu([cs, mean_k ns]@W_g1)@W_g2); out = sum_k g_k * h_k

Strategy (pure data parallel, Bc=4096 rows/core on 8 cores):
  - ns ships twice in fp8e4m3: token-major (m8, aggregation contraction)
    and host-pretransposed d-major (nsd) - no on-device DMA transpose.
  - msg is ONE fp8 DoubleRow matmul per 128-token block (13 blocks per
    64-row chunk): k-tile 0 contracts ns dims against W2; k-tile 1
    contracts a one-hot row-selector constant against t8 = cs@W1+b_msg
    (host-computed, shipped fp8 interleaved with W2), folding the
    per-row broadcast add into the same instruction at 0.5 cyc/row.
  - 128-token psum packing => relu evacuation is one whole-chunk
    [128,832] op alternating between DVE and Activation per chunk.
  - aggregation matmuls write (mask,row)-compact psum; one copy per
    chunk moves all four aggregates to SBUF (engine opposite the evac).
    The dense 1/26 gating-mean mask is a static bpack constant. fse
    (masked mean of msg) accumulates 13 per-block slabs start=False
    onto a matmul-zeroed region (PSUM start=True zeroes a whole 2KB
    bank, so split rows cannot accumulate across start groups).
  - two-phase chunk pipeline: fse+copy of chunk c-1 are emitted after
    msg/agg/evac of chunk c, hiding the evac latency; the per-super
    expert/gating chain (b-split [128,256] block-diagonal weights)
    software-pipelines across supers; epilogue gating runs on GPSIMD
    from an evacuated SBUF bounce; startup DMAs are ordered so the
    first msg matmul's inputs clear the serial HWDGE queue first.
"""

import os
import sys

sys.path.insert(0, "/opt/trn_rl_repo")

import numpy as np
import ml_dtypes

import concourse.bass as bass
import concourse.mybir as mybir
import concourse.tile as tile
from concourse.ap import AP
from concourse.bass_utils import run_bass_kernel_spmd

F32 = mybir.dt.float32
BF16 = mybir.dt.bfloat16
F8E4 = mybir.dt.float8e4

D = 64
K = 26
NCORES = 8
CB = 64            # b-rows per chunk
SB = 512           # b-rows per super (8 chunks)
GC = 4             # chunks per DMA group
NBLK = 13          # 128-token blocks per chunk (13*128 == 64*26)
NTOK = 1664        # tokens per chunk
AFT = mybir.ActivationFunctionType
ALU = mybir.AluOpType
AXL = mybir.AxisListType
DR = mybir.MatmulPerfMode.DoubleRow

NSF = 1024         # m8: 16 tiles * 64 dims per chunk
EVAC_DVE = {0, 2, 4, 6}  # chunks whose evac runs on DVE (copy on Act)
MKA = 128          # mkfa cols per chunk (16 tiles x (2 masks x 4 rows))
MKF = 76           # mkff cols per chunk (per-block row slabs, overlap at fins)

# wpack (bf16) column layout: 9 block-diagonal [128,128] weights,
# 2 [128,64] gating blocks, wg2blk, bias cols (same as v2)
WB = {
    "wl1": 0, "wlb": 128, "wu1": 256, "wub": 384, "wf": 512, "wfs": 640,
    "wda": 768, "wdb": 896, "wds": 1024,
}
WG1A = 1152
WG1B = 1216
WG2 = 1280
WPC = 1322

# bpack (bf16) column layout
BW1B = 0           # w1b65 [0:65, 0:64]
BI64 = 64          # identity stacked twice [128, 64:128]
BIP1 = 128         # 0.1*identity stacked twice [128, 128:192]
BEY3 = 192         # eye3 at rows 64:67, cols 192:195
BSMN = 196         # static mean mask [0:104, 196:200]
BZ = 200           # guaranteed-zero region [128, 200:264]
BPC = 264


def _blocks13():
    """Static row structure of the 13 128-token blocks of one chunk."""
    out = []
    for j in range(NBLK):
        t0, t1 = 128 * j, 128 * j + 128
        rlo = -((-t0) // K)
        rhi = -((-t1) // K)
        has_fin = (t0 % K) != 0
        out.append((rlo, rhi, has_fin, t0 // K))
    return out


BLK = _blocks13()


def _blocksf():
    """Per-block fse slab: (first row incl. fin, end row, col offset)."""
    out, o = [], 0
    for j in range(NBLK):
        rlo, rhi, has_fin, sj = BLK[j]
        a0 = sj if has_fin else rlo
        out.append((a0, rhi, o))
        o += rhi - a0
    assert o == MKF
    return out


BLKF = _blocksf()


def r32(ap):
    return ap.bitcast(mybir.dt.float32r)


def _split_waits(nc):
    """Walrus encodes at most one sync-wait command on most TPB instructions.
    Hoist extra on_wait entries into standalone single-wait EventSemaphore
    instructions placed immediately before, on the same engine queue."""
    for f in nc.m.functions:
        for blk in f.blocks:
            insts = list(blk.instructions)
            out = []
            changed = False
            for inst in insts:
                si = inst.sync_info
                ow = list(si.on_wait) if (si is not None and si.on_wait) else []
                if len(ow) > 1:
                    changed = True
                    for w in ow[:-1]:
                        out.append(
                            mybir.InstEventSemaphore(
                                name=nc.get_next_instruction_name(),
                                engine=inst.engine,
                                ins=[],
                                outs=[],
                                sync_info=mybir.SyncInfo(on_wait=[w], on_update=[]),
                            )
                        )
                    inst.sync_info = mybir.SyncInfo(
                        on_wait=[ow[-1]], on_update=list(si.on_update or [])
                    )
                out.append(inst)
            if changed:
                blk.instructions = out


def build_program(Bc, zb=True):
    assert Bc % SB == 0
    nsup = Bc // SB
    nch = Bc // CB
    ngr = nch // GC

    nc = bass.Bass(trn_type="TRN2", target_bir_lowering=False, debug=False)

    m8_d = nc.dram_tensor("m8", [ngr, 104, GC * NSF], F8E4, kind="ExternalInput").ap()
    nsd_d = nc.dram_tensor("nsd", [D, nch * NTOK], F8E4, kind="ExternalInput").ap()
    selc_d = nc.dram_tensor("selc", [D, NTOK], F8E4, kind="ExternalInput").ap()
    t8f_d = nc.dram_tensor("t8f", [nsup, D, 1024], F8E4, kind="ExternalInput").ap()
    mkc_d = nc.dram_tensor("mkc", [nsup, 128, 8 * (MKA + MKF)], BF16, kind="ExternalInput").ap()
    cst2_d = nc.dram_tensor("cst2", [128, Bc // 2], BF16, kind="ExternalInput").ap()
    wpack_d = nc.dram_tensor("wpack", [128, WPC], BF16, kind="ExternalInput").ap()
    fpack_d = nc.dram_tensor("fpack", [128, 8], F32, kind="ExternalInput").ap()
    bpack_d = nc.dram_tensor("bpack", [128, BPC], BF16, kind="ExternalInput").ap()
    out_d = nc.dram_tensor("out", [Bc, D], BF16, kind="ExternalOutput").ap()

    out_v = out_d.rearrange("(s w p) d -> s p w d", p=128, w=4)

    NSB = 4 * NTOK          # ns cols per group in the nsb tile
    SELO = NSB              # sel region col offset
    NSBW = NSB + NTOK       # nsb tile width

    with tile.TileContext(nc) as tc:
        with (
            tc.tile_pool(name="sing", bufs=1) as sing,
            tc.tile_pool(name="pmb", bufs=8) as pmb,
            tc.tile_pool(name="pmsgr", bufs=6) as pmsgr,
            tc.tile_pool(name="psup", bufs=8) as psup,
            tc.tile_pool(name="pch", bufs=5) as pch,
            tc.tile_pool(name="pot", bufs=10) as pot,
            tc.tile_pool(name="pstg", bufs=6) as pstg,
            tc.tile_pool(name="ppm", bufs=2, space="PSUM") as ppm,
            tc.tile_pool(name="ppag", bufs=1, space="PSUM") as ppag,
            tc.tile_pool(name="ppsc", bufs=2, space="PSUM") as ppsc,
            tc.tile_pool(name="ppe", bufs=1, space="PSUM") as ppe,
        ):
            # startup order: the first msg matmul needs selc + t8f + nsd0 +
            # m80 (HWDGE is serial at ~630ns/DMA, so critical items go first;
            # wpack/fpack/mkc ride the parallel SWDGE path)
            nsb_bufs = [sing.tile([D, NSBW], F8E4, tag=f"nsb{i}", name=f"nsb{i}") for i in range(4)]
            nc.scalar.dma_start(out=nsb_bufs[0][:, SELO:SELO + NTOK], in_=selc_d)
            init_done = {0}
            t8b0 = psup.tile([D, 1024], F8E4, tag="t8")
            nc.sync.dma_start(out=t8b0, in_=t8f_d[0])
            bpack = sing.tile([128, BPC], BF16, tag="bpack")
            nc.gpsimd.dma_start(out=bpack, in_=bpack_d)
            cst20 = psup.tile([128, 256], BF16, tag="cst2")
            wpack = sing.tile([128, WPC], BF16, tag="wpack")
            fpack = sing.tile([128, 8], F32, tag="fpack")
            nc.gpsimd.dma_start(out=wpack, in_=wpack_d)
            nc.gpsimd.dma_start(out=fpack, in_=fpack_d)

            i64a = bpack[0:64, BI64:BI64 + 64]
            i64b = bpack[64:128, BI64:BI64 + 64]
            ip1a = bpack[0:64, BIP1:BIP1 + 64]
            ip1b = bpack[64:128, BIP1:BIP1 + 64]
            ey3a = bpack[64:67, BEY3:BEY3 + 3]
            smean = bpack[0:104, BSMN:BSMN + 4]

            # startup observer matmul so later matmuls carry <=1 wait
            pwu = ppsc.tile([128, 512], F32, tag="psc")
            nc.tensor.matmul(pwu[0:32, 0:32], bpack[0:32, 0:32], bpack[0:32, 0:32], start=True, stop=True)

            def emit_front(s, h, nsb, t8b):
                cc = h % GC

                # ---- fused msg matmuls: ns@W2 + onehot@t8 per 128-tok block
                pm = ppm.tile([128, 832], F32, tag="pm")
                rhs = t8b.rearrange("p (h j d) -> p h j d", h=8, j=2)[0:D, h]
                for j in range(NBLK):
                    lhsT = AP(
                        nsb.tensor,
                        nsb.offset + NTOK * cc + 128 * j,
                        [list(nsb.ap[0][:]), [SELO - NTOK * cc, 2], [1, 128]],
                    )
                    nc.tensor.matmul(
                        pm[:, 64 * j:64 * j + 64], lhsT, rhs,
                        start=True, stop=True, perf_mode=DR,
                    )

                # ---- relu evacuation (whole chunk, engine by table) ----
                msgr = pmsgr.tile([128, 832], BF16, tag="msgr")
                if h in EVAC_DVE:
                    nc.vector.tensor_scalar(msgr, pm, 0.0, None, ALU.max)
                else:
                    nc.scalar.activation(msgr, pm, AFT.Relu)
                return msgr

            def emit_back(s, h, mbl, mkfa, mkff, agg_all, msgr):
                cc = h % GC
                P0, P1 = (0, 64) if h < 4 else (64, 128)
                hw_ = h % 4
                c0 = cc * NSF             # m8 col base for chunk
                ma0 = h * MKA             # mkfa col base
                mf0 = h * MKF             # mkff col base
                pgf = ppag.tile([128, 512], F32, tag="pagg", bufs=1)
                pagg = pgf[:, 256 * (h % 2):256 * (h % 2) + 256]

                # ---- aggregation matmuls (tier0/tier2 + static mean) ----
                for q in range(16):
                    lh = mbl[:, c0 + 64 * q: c0 + 64 * (q + 1)]
                    for m in range(2):
                        nc.tensor.matmul(
                            pagg[P0:P1, 64 * m + 4 * q:64 * m + 4 * q + 4],
                            lh,
                            mkfa[:, ma0 + 8 * q + 4 * m: ma0 + 8 * q + 4 * m + 4],
                            start=True, stop=True,
                        )
                    nc.tensor.matmul(
                        pagg[P0:P1, 128 + 4 * q:132 + 4 * q],
                        lh, smean,
                        start=True, stop=True,
                    )

                # ---- fse matmuls (masked mean of msg over tier==1) ----
                # start=True zeroes the whole 2KB psum bank, so rows split
                # across blocks cannot accumulate across start-groups: zero
                # the region once, then accumulate all 13 blocks start=False.
                zrow = bpack[0:1, BZ:BZ + 64]
                nc.tensor.matmul(
                    pagg[P0:P1, 192:256], zrow, zrow,
                    start=True, stop=False, skip_group_check=True,
                )
                for j in range(NBLK):
                    a0, rhi, o_j = BLKF[j]
                    nc.tensor.matmul(
                        pagg[P0:P1, 192 + a0:192 + rhi],
                        msgr[:, 64 * j:64 * (j + 1)],
                        mkff[:, mf0 + o_j:mf0 + o_j + rhi - a0],
                        start=False, stop=(j == NBLK - 1),
                        skip_group_check=True,
                    )

                # ---- one copy per chunk: psum aggregates -> agg_all ----
                agv = agg_all.rearrange("p (m b) -> p m b", m=4)
                dst = agv[P0:P1, :, 64 * hw_:64 * hw_ + 64]
                srcc = pagg[P0:P1].rearrange("p (m b) -> p m b", m=4)
                if h in EVAC_DVE:
                    nc.scalar.activation(dst, srcc, AFT.Copy)
                else:
                    nc.vector.tensor_copy(dst, srcc)

            W = lambda k: wpack[:, WB[k]:WB[k] + 128]

            def emit_chain(s, cst2, agg_all):
                """Generator: chain+epilogue for super s (b-split [128,256])."""
                agl = agg_all[:, 0:256]
                agd = agg_all[:, 256:512]
                agm = agg_all[:, 512:768]
                aggf2 = agg_all[:, 768:1024]

                # step 1: everything that only needs the aggregates
                pzg = ppsc.tile([128, 512], F32, tag="psc")
                nc.tensor.matmul(pzg[0:64, 0:256], wpack[:, WG1A:WG1A + 64], cst2, start=True, stop=False)
                nc.tensor.matmul(pzg[0:64, 0:256], wpack[:, WG1B:WG1B + 64], agm, start=False, stop=True)
                g1r = pch.tile([64, 256], BF16, tag="g1r")
                nc.vector.tensor_scalar(
                    g1r, pzg[0:64, 0:256], fpack[0:64, 0:1], 0.0,
                    ALU.add, ALU.max,
                )
                ph = ppsc.tile([128, 512], F32, tag="psc")
                nc.tensor.matmul(ph[:, 0:256], W("wl1"), cst2, start=True, stop=False)
                nc.tensor.matmul(ph[:, 0:256], W("wlb"), agl, start=False, stop=True)
                nc.tensor.matmul(ph[:, 256:512], W("wu1"), cst2, start=True, stop=False)
                nc.tensor.matmul(ph[:, 256:512], W("wub"), aggf2, start=False, stop=True)
                hlu = pch.tile([128, 512], BF16, tag="hlu")
                if zb:
                    nc.scalar.activation(hlu, ph, AFT.Tanh)
                else:
                    nc.scalar.activation(hlu[:, 0:256], ph[:, 0:256], AFT.Tanh, bias=fpack[:, 2:3])
                    nc.scalar.activation(hlu[:, 256:512], ph[:, 256:512], AFT.Tanh, bias=fpack[:, 3:4])
                hl = hlu[:, 0:256]
                hu0 = hlu[:, 256:512]
                yield

                # step 2: logits (g1r ready by now) + tfd0 d-side (ready)
                pex = pzg[:, 384:400]
                exw = pot.tile([128, 12], F32, tag="exw")
                if zb:
                    for w in range(4):
                        lo = 32 * (w >= 2)
                        nc.tensor.matmul(
                            pex[:, 3 * w:3 * w + 3],
                            g1r[lo:lo + 32, 128 * (w % 2):128 * (w % 2) + 128],
                            wpack[lo:lo + 32, WG2 + 3 * (w >= 2):WG2 + 3 * (w >= 2) + 3],
                            start=True, stop=True,
                        )
                    nc.scalar.activation(exw, pex[:, 0:12], AFT.Exp)
                else:
                    nc.tensor.matmul(pzg[64:67, 0:256], wpack[0:64, WG2:WG2 + 3], g1r, start=True, stop=True)
                    nc.tensor.matmul(pzg[64:67, 256:512], wpack[0:64, WG2 + 3:WG2 + 6], g1r, start=True, stop=True)
                    expt = pch.tile([128, 512], BF16, tag="expt")
                    nc.scalar.activation(
                        expt[64:67], pzg[64:67, 0:512], AFT.Exp,
                        bias=fpack[64:67, 1:2],
                    )
                    for w in range(4):
                        eb = 256 * (w >= 2) + 128 * (w % 2)
                        nc.tensor.matmul(pex[:, 3 * w:3 * w + 3], expt[64:67, eb:eb + 128], ey3a, start=True, stop=True)
                    nc.vector.tensor_copy(exw, pex[:, 0:12])
                pz0 = ppsc.tile([128, 512], F32, tag="psc")
                nc.tensor.matmul(pz0[:, 256:512], W("wda"), cst2, start=True, stop=False)
                nc.tensor.matmul(pz0[:, 256:512], W("wdb"), agd, start=False, stop=True)
                yield

                # step 3: tfd0 f-side (waits hlu act) + act + softmax scalars
                nc.tensor.matmul(pz0[:, 0:256], W("wf"), hu0, start=True, stop=True)
                tfd0 = pch.tile([128, 512], BF16, tag="tfd0")
                if zb:
                    nc.scalar.activation(tfd0, pz0, AFT.Tanh)
                else:
                    nc.scalar.activation(tfd0[:, 0:256], pz0[:, 0:256], AFT.Tanh, bias=fpack[:, 4:5])
                    nc.scalar.activation(tfd0[:, 256:512], pz0[:, 256:512], AFT.Tanh, bias=fpack[:, 5:6])
                se = pot.tile([128, 4], F32, tag="se")
                nc.vector.tensor_reduce(
                    se.rearrange("p (w o) -> p w o", o=1),
                    exw.rearrange("p (w k) -> p w k", k=3),
                    AXL.X, ALU.add,
                )
                rc = pot.tile([128, 4], F32, tag="rc")
                nc.vector.reciprocal(rc, se)
                gk = pot.tile([128, 12], F32, tag="gk")
                for w in range(4):
                    nc.gpsimd.tensor_scalar(
                        gk[:, 3 * w:3 * w + 3], exw[:, 3 * w:3 * w + 3],
                        rc[:, w:w + 1], None, ALU.mult,
                    )
                yield

                # steps 4-5: tfd1 (d-side ready mms first, tfd0-waiters later)
                pz1 = ppsc.tile([128, 512], F32, tag="psc")
                nc.tensor.matmul(pz1[:, 256:512], W("wda"), cst2, start=True, stop=False)
                nc.tensor.matmul(pz1[:, 256:512], W("wdb"), agd, start=False, stop=False)
                yield
                nc.tensor.matmul(pz1[:, 256:512], W("wds"), tfd0[:, 256:512], start=False, stop=True)
                nc.tensor.matmul(pz1[:, 0:256], W("wf"), hu0, start=True, stop=False)
                nc.tensor.matmul(pz1[:, 0:256], W("wfs"), tfd0[:, 0:256], start=False, stop=True)
                tfd1 = pch.tile([128, 512], BF16, tag="tfd1")
                if zb:
                    nc.scalar.activation(tfd1, pz1, AFT.Tanh)
                else:
                    nc.scalar.activation(tfd1[:, 0:256], pz1[:, 0:256], AFT.Tanh, bias=fpack[:, 4:5])
                    nc.scalar.activation(tfd1[:, 256:512], pz1[:, 256:512], AFT.Tanh, bias=fpack[:, 5:6])
                yield

                # steps 6-7: tfd2
                pz2 = ppsc.tile([128, 512], F32, tag="psc")
                nc.tensor.matmul(pz2[:, 256:512], W("wda"), cst2, start=True, stop=False)
                nc.tensor.matmul(pz2[:, 256:512], W("wdb"), agd, start=False, stop=False)
                nc.tensor.matmul(pz2[:, 256:512], W("wds"), tfd0[:, 256:512], start=False, stop=False)
                yield
                nc.tensor.matmul(pz2[:, 256:512], W("wds"), tfd1[:, 256:512], start=False, stop=True)
                nc.tensor.matmul(pz2[:, 0:256], W("wf"), hu0, start=True, stop=False)
                nc.tensor.matmul(pz2[:, 0:256], W("wfs"), tfd0[:, 0:256], start=False, stop=False)
                nc.tensor.matmul(pz2[:, 0:256], W("wfs"), tfd1[:, 0:256], start=False, stop=True)
                tfd2 = pch.tile([128, 512], BF16, tag="tfd2")
                if zb:
                    nc.scalar.activation(tfd2, pz2, AFT.Tanh)
                else:
                    nc.scalar.activation(tfd2[:, 0:256], pz2[:, 0:256], AFT.Tanh, bias=fpack[:, 4:5])
                    nc.scalar.activation(tfd2[:, 256:512], pz2[:, 256:512], AFT.Tanh, bias=fpack[:, 5:6])
                tfds = [tfd0, tfd1, tfd2]
                yield

                stg = pstg.tile([128, 256], BF16, tag="stg")
                for pair in range(2):
                    pef = ppe.tile([128, 512], F32, tag="pe", bufs=1)
                    pes = []
                    for w in (2 * pair, 2 * pair + 1):
                        hi = w >= 2
                        R0, R1 = (64, 128) if hi else (0, 64)
                        b0 = 128 * (w % 2)
                        i64 = i64b if hi else i64a
                        ip1 = ip1b if hi else ip1a
                        sl = slice(b0, b0 + 128)
                        pe = pef[:, 256 * (w % 2):256 * (w % 2) + 192]
                        pes.append(pe)
                        nc.tensor.matmul(pe[:, 0:64], hl[R0:R1, sl], i64, start=True, stop=True)
                        nc.tensor.matmul(pe[:, 64:128], hu0[R0:R1, sl], i64, start=True, stop=False)
                        nc.tensor.matmul(pe[:, 64:128], tfds[0][R0:R1, sl], ip1, start=False, stop=False)
                        nc.tensor.matmul(pe[:, 64:128], tfds[1][R0:R1, sl], ip1, start=False, stop=False)
                        nc.tensor.matmul(pe[:, 64:128], tfds[2][R0:R1, sl], ip1, start=False, stop=True)
                        nc.tensor.matmul(pe[:, 128:192], cst2[R0:R1, sl], i64, start=True, stop=False)
                        nc.tensor.matmul(pe[:, 128:192], tfds[0][R0:R1, 256 + b0:256 + b0 + 128], ip1, start=False, stop=False)
                        nc.tensor.matmul(pe[:, 128:192], tfds[1][R0:R1, 256 + b0:256 + b0 + 128], ip1, start=False, stop=False)
                        nc.tensor.matmul(pe[:, 128:192], tfds[2][R0:R1, 256 + b0:256 + b0 + 128], ip1, start=False, stop=True)
                    pevs = []
                    for i, w in enumerate((2 * pair, 2 * pair + 1)):
                        pev = pot.tile([128, 192], F32, tag="pev")
                        if i == 0:
                            nc.vector.tensor_copy(pev, pes[i])
                        else:
                            nc.scalar.activation(pev, pes[i], AFT.Copy)
                        pevs.append(pev)
                    for i, w in enumerate((2 * pair, 2 * pair + 1)):
                        pev = pevs[i]
                        t1 = pot.tile([128, 64], F32, tag="t1")
                        nc.gpsimd.tensor_scalar(t1, pev[:, 0:64], gk[:, 3 * w:3 * w + 1], None, ALU.mult)
                        t2 = pot.tile([128, 64], F32, tag="t2")
                        nc.gpsimd.tensor_scalar(t2, pev[:, 64:128], gk[:, 3 * w + 1:3 * w + 2], None, ALU.mult)
                        t3 = pot.tile([128, 64], F32, tag="t3")
                        nc.gpsimd.tensor_scalar(t3, pev[:, 128:192], gk[:, 3 * w + 2:3 * w + 3], None, ALU.mult)
                        nc.gpsimd.tensor_tensor(t1, t1, t2, ALU.add)
                        nc.gpsimd.tensor_tensor(stg[:, 64 * w:64 * w + 64], t1, t3, ALU.add)
                    yield
                nc.sync.dma_start(out=out_v[s], in_=stg.rearrange("p (w d) -> p w d", w=4))

            def advance(gen, n):
                if gen is None:
                    return None
                for _ in range(n):
                    try:
                        next(gen)
                    except StopIteration:
                        return None
                return gen

            pending = None
            t8b = cst2 = mkfa = mkff = agg_all = None
            mbl = nsb = None
            backq = []             # deferred back-phase args (depth 2)
            sup_state = {}
            for slot in range(nch + 1):
                s, idx = slot // 8, slot % 8
                if idx == 0 and s < nsup:
                    # per-super state (mkc/t8/cst2 prefetched a super ahead)
                    if s == 0:
                        t8b, cst2 = t8b0, cst20
                        mkc = psup.tile([128, 8 * (MKA + MKF)], BF16, tag="mkc")
                        nc.gpsimd.dma_start(out=mkc, in_=mkc_d[0])
                    else:
                        t8b, cst2, mkc = t8b_next, cst2_next, mkc_next
                    mkfa = mkc[0:104, 0:8 * MKA]
                    mkff = mkc[:, 8 * MKA:8 * (MKA + MKF)]
                    agg_all = psup.tile([128, 1024], BF16, tag="agg_all")
                    sup_state[s] = (cst2, agg_all, mkff)
                if idx % GC == 0 and s < nsup:
                    # per-group loads
                    g = slot // GC
                    nsb = nsb_bufs[g % 4]
                    if g % 4 not in init_done:
                        init_done.add(g % 4)
                        nc.gpsimd.dma_start(out=nsb[:, SELO:SELO + NTOK], in_=selc_d)
                    mbl = pmb.tile([104, GC * NSF], F8E4, tag="mbl")
                    if g == 0:
                        # chunk-0 slices first so compute starts early
                        nc.sync.dma_start(out=nsb[:, 0:NTOK], in_=nsd_d[:, 0:NTOK])
                        nc.sync.dma_start(out=mbl[:, 0:NSF], in_=m8_d[0][:, 0:NSF])
                        nc.sync.dma_start(out=nsb[:, NTOK:NSB], in_=nsd_d[:, NTOK:NSB])
                        nc.sync.dma_start(out=mbl[:, NSF:], in_=m8_d[0][:, NSF:])
                    else:
                        hb = NSB // 2
                        nc.sync.dma_start(out=nsb[:, 0:hb], in_=nsd_d[:, g * NSB:g * NSB + hb])
                        nc.sync.dma_start(out=mbl[:, 0:2 * NSF], in_=m8_d[g][:, 0:2 * NSF])
                        nc.sync.dma_start(out=nsb[:, hb:NSB], in_=nsd_d[:, g * NSB + hb:(g + 1) * NSB])
                        nc.sync.dma_start(out=mbl[:, 2 * NSF:], in_=m8_d[g][:, 2 * NSF:])
                if slot == 1:
                    nc.sync.dma_start(out=cst20, in_=cst2_d[:, 0:256])
                if idx == 4 and s + 1 < nsup:
                    t8b_next = psup.tile([D, 1024], F8E4, tag="t8")
                    nc.sync.dma_start(out=t8b_next, in_=t8f_d[s + 1])
                    mkc_next = psup.tile([128, 8 * (MKA + MKF)], BF16, tag="mkc")
                    nc.gpsimd.dma_start(out=mkc_next, in_=mkc_d[s + 1])
                    cst2_next = psup.tile([128, 256], BF16, tag="cst2")
                    nc.sync.dma_start(out=cst2_next, in_=cst2_d[:, (s + 1) * 256:(s + 2) * 256])
                if slot < nch:
                    h = idx
                    msgr = emit_front(s, h, nsb, t8b)
                    pending = advance(pending, 1)
                    if len(backq) == 2:
                        emit_back(*backq.pop(0))
                    pending = advance(pending, 1)
                    backq.append((s, h, mbl, mkfa, mkff, agg_all, msgr))
                else:
                    while backq:
                        emit_back(*backq.pop(0))
                if idx == 1 and s > 0:
                    # super s-1 fully emitted (its last back ran this slot)
                    advance(pending, 99)
                    c2p, aap, _ = sup_state.pop(s - 1)
                    pending = emit_chain(s - 1, c2p, aap)
            advance(pending, 99)
            c2p, aap, _ = sup_state.pop(nsup - 1)
            pending = emit_chain(nsup - 1, c2p, aap)
            advance(pending, 99)
    return nc


# ---------------- host-side packing ----------------

def host_pack(inputs, core, Bc):
    b0 = core * Bc
    ns = np.asarray(inputs["neighbor_states"][b0:b0 + Bc], np.float32)
    cs = np.asarray(inputs["current_state"][b0:b0 + Bc], np.float32)
    tier = np.asarray(inputs["tier"][b0:b0 + Bc], np.int32)

    nch = Bc // CB
    ngr = nch // GC
    nsup = Bc // SB

    w_mask = []
    for t in (0, 2, 1):
        m = (tier == t)
        w_mask.append(m.astype(np.float32) / np.maximum(m.sum(-1, keepdims=True), 1.0))
    w0, w2m, w1 = w_mask  # local, dist, func

    # token-major fp8 for aggregation (16 tiles of 4 rows x 26 k per chunk)
    nsr = ns.reshape(nch, 16, 4, K, D)
    ns_tok = np.ascontiguousarray(nsr.transpose(0, 2, 3, 1, 4)).reshape(nch, 104, NSF)
    m8 = np.ascontiguousarray(
        ns_tok.astype(ml_dtypes.float8_e4m3).reshape(ngr, GC, 104, NSF).transpose(0, 2, 1, 3)
    ).reshape(ngr, 104, GC * NSF)

    # d-major fp8 for the msg matmul: col = chunk*1664 + 26*row + k
    nsd = np.ascontiguousarray(
        ns.reshape(nch, NTOK, D).transpose(2, 0, 1)
    ).reshape(D, nch * NTOK).astype(ml_dtypes.float8_e4m3)

    # one-hot row selector [64, 1664]: sel[r, T] = (T//26 == r)
    T = np.arange(NTOK)
    selc = (T[None, :] // K == np.arange(D)[:, None]).astype(ml_dtypes.float8_e4m3)

    # mkfa: per chunk [104, 16*(2,4)] block-diagonal tier0/tier2 weights
    wm2 = np.stack([w0, w2m], 0).reshape(2, nch, 16, 4, K)  # [m,c,q,g,k]
    M = np.zeros((nch, 4, K, 16, 2, 4), np.float32)
    for g in range(4):
        M[:, g, :, :, :, g] = wm2[:, :, :, g, :].transpose(1, 3, 2, 0)
    mkfa = M.reshape(nch, 104, MKA)
    mkfa = np.ascontiguousarray(
        mkfa.reshape(nsup, 8, 104, MKA).transpose(0, 2, 1, 3)
    ).reshape(nsup, 104, 8 * MKA)

    # mkff: per chunk [128, 76]; cols 0..63 fresh-row weights, 64..75 fins
    w1r = w1.reshape(nch, 64, K)
    mkff = np.zeros((nch, 128, MKF), np.float32)
    for j in range(NBLK):
        a0, rhi, o_j = BLKF[j]
        for r in range(a0, rhi):
            tA = max(K * r, 128 * j)
            tB = min(K * r + K, 128 * j + 128)
            ts = np.arange(tA, tB) - 128 * j
            mkff[:, ts, o_j + r - a0] = w1r[:, r, tA - K * r:tB - K * r]
    mkff = np.ascontiguousarray(
        mkff.reshape(nsup, 8, 128, MKF).transpose(0, 2, 1, 3)
    ).reshape(nsup, 128, 8 * MKF)
    mkc = np.zeros((nsup, 128, 8 * (MKA + MKF)), np.float32)
    mkc[:, 0:104, 0:8 * MKA] = mkfa
    mkc[:, :, 8 * MKA:] = mkff
    mkc = mkc.astype(ml_dtypes.bfloat16)

    cst = cs.T.astype(np.float32)                      # [64, Bc]
    cst2 = np.ascontiguousarray(
        cst.reshape(D, nsup, 2, 256).transpose(2, 0, 1, 3)
    ).reshape(128, nsup * 256).astype(ml_dtypes.bfloat16)

    def blk(w):
        z = np.zeros((128, 128), np.float32)
        z[0:64, 0:64] = w
        z[64:128, 64:128] = w
        return z

    wl = np.asarray(inputs["W_local"], np.float32)
    wu = np.asarray(inputs["W_upd"], np.float32)
    wf = np.asarray(inputs["W_fcnf"], np.float32)
    wd = np.asarray(inputs["W_dcnf"], np.float32)
    wg1 = np.asarray(inputs["W_g1"], np.float32)
    wg2 = np.asarray(inputs["W_g2"], np.float32)
    wmsg = np.asarray(inputs["W_msg"], np.float32)

    wpack = np.zeros((128, WPC), np.float32)
    for k, w in [("wl1", wl[:D]), ("wlb", wl[D:]), ("wu1", wu[:D]), ("wub", wu[D:]),
                 ("wf", wf), ("wfs", 0.1 * wf), ("wda", wd[:D]), ("wdb", wd[D:]),
                 ("wds", 0.1 * wd[:D])]:
        wpack[:, WB[k]:WB[k] + 128] = blk(w)
    g1a = np.zeros((128, 64), np.float32)
    g1a[0:64, 0:32] = wg1[:D]; g1a[64:128, 32:64] = wg1[:D]
    g1b = np.zeros((128, 64), np.float32)
    g1b[0:64, 0:32] = wg1[D:]; g1b[64:128, 32:64] = wg1[D:]
    wpack[:, WG1A:WG1A + 64] = g1a
    wpack[:, WG1B:WG1B + 64] = g1b
    wpack[0:32, WG2:WG2 + 3] = wg2
    wpack[32:64, WG2 + 3:WG2 + 6] = wg2
    fpack = np.zeros((128, 8), np.float32)
    bg1 = np.asarray(inputs["b_g1"], np.float32)
    fpack[0:32, 0] = bg1; fpack[32:64, 0] = bg1
    fpack[64:67, 1] = np.asarray(inputs["b_g2"], np.float32)
    bl = np.asarray(inputs["b_local"], np.float32)
    bu = np.asarray(inputs["b_upd"], np.float32)
    bfc = np.asarray(inputs["b_fcnf"], np.float32)
    bdc = np.asarray(inputs["b_dcnf"], np.float32)
    fpack[0:64, 2] = bl; fpack[64:128, 2] = bl
    fpack[0:64, 3] = bu; fpack[64:128, 3] = bu
    fpack[0:64, 4] = bfc; fpack[64:128, 4] = bfc
    fpack[0:64, 5] = bdc; fpack[64:128, 5] = bdc

    bpack = np.zeros((128, BPC), np.float32)
    bpack[0:64, BW1B:BW1B + 64] = wmsg[:D]
    bpack[64, BW1B:BW1B + 64] = np.asarray(inputs["b_msg"], np.float32)
    ii = np.eye(64, dtype=np.float32)
    bpack[0:64, BI64:BI64 + 64] = ii; bpack[64:128, BI64:BI64 + 64] = ii
    bpack[0:64, BIP1:BIP1 + 64] = 0.1 * ii; bpack[64:128, BIP1:BIP1 + 64] = 0.1 * ii
    e3 = np.eye(3, dtype=np.float32)
    bpack[64:67, BEY3:BEY3 + 3] = e3
    # static mean mask [104, 4]: token (g,k) -> col g, value 1/26
    tg = np.arange(104) // K
    bpack[np.arange(104), BSMN + tg] = 1.0 / K

    # t8f: per-super rhs tile [64, (h,2,64)]: W2 at (h,0), cs@W1+b at (h,1)
    t8 = cs @ wmsg[:D] + np.asarray(inputs["b_msg"], np.float32)   # [Bc, 64]
    t8f = np.zeros((nsup, D, 8, 2, D), np.float32)
    t8f[:, :, :, 0, :] = wmsg[D:][:, None, :]
    t8f[:, :, :, 1, :] = np.ascontiguousarray(
        t8.reshape(nsup, 8, D, D).transpose(0, 2, 1, 3))
    t8f = t8f.reshape(nsup, D, 1024).astype(ml_dtypes.float8_e4m3)

    m = {
        "m8": m8,
        "nsd": nsd,
        "selc": selc,
        "t8f": t8f,
        "mkc": mkc,
        "cst2": cst2,
        "wpack": wpack.astype(ml_dtypes.bfloat16),
        "fpack": fpack,
        "bpack": bpack.astype(ml_dtypes.bfloat16),
    }
    return {k: np.ascontiguousarray(v) for k, v in m.items()}


def _zb(inputs):
    return all(
        not np.any(np.asarray(inputs[k]))
        for k in ("b_local", "b_upd", "b_fcnf", "b_dcnf", "b_g2")
    )


_CACHE = {}


def _get_program(Bc, zb=True):
    key = (Bc, zb)
    if key not in _CACHE:
        nc = build_program(Bc, zb=zb)
        _split_waits(nc)
        _CACHE[key] = nc
    return _CACHE[key]


def run(inputs, trace=False):
    B = inputs["current_state"].shape[0]
    Bc = B // NCORES
    nc = _get_program(Bc, zb=_zb(inputs))
    in_maps = [host_pack(inputs, core, Bc) for core in range(NCORES)]
    res = run_bass_kernel_spmd(
        nc, in_maps, core_ids=list(range(NCORES)), trace=trace
    )
    out = np.concatenate([np.asarray(r["out"]).astype(np.float32) for r in res.results], axis=0)
    return out, res


def kernel(**inputs):
    out, _ = run(inputs)
    return out
